# revision 93
# baseline (speedup 1.0000x reference)
"""DigitCapsule routing kernel for 8 TRN2 NeuronCores.

Math (reference):
    u_hat[b,r,c,o] = sum_i W[r,c,o,i] x[b,c,i]
    b=0; 3 iterations of: c=softmax_r(b); s=sum_r c*u_hat; v=squash(s);
                          b += sum_o u_hat*v
    returns v (B, C, OC)

Key restructure -- u_hat (536MB) is never materialized.  Each routing
iteration is expressed as matmuls against W directly:
  - logits:  b_inc[b,r,c] = sum_oi W[r,c,oi] * M[b,c,oi],  M = x (x) v
             (contract oi=256; needs W transposed so oi is on partitions)
  - weights: p = exp(b)  (no max subtraction needed, |b| stays small)
  - sums:    G[b,c,oi] = sum_r p[b,r,c] W[r,c,oi]  (contract r; natural W)
             S[b,c,o]  = sum_i G[b,c,o,i] x[b,c,i];  Z[b,c] = sum_r p
  - s = S/Z; v = squash(s)
Iteration 0 has p uniform, so s0/v0/M0 depend only on Wsum = sum_r W --
computed host-side in fp32 (exact) and fed in as the tiny M0 tensor.
The device runs only routing iterations 1 and 2.

Precision: the logits are exponentiated, so every path feeding them must
stay f16-or-better (bf16 W gives |db|~0.03 which compounds to ~30%
output error).

Sharding: R=16384 split over 8 cores (2048 each).  Cross-core traffic is
2 packed AllReduces (34KB each).  W is fed to each core in two layouts
(natural [r, (c,oi)] and transposed [oi, (c,r)]), both prepared
host-side so every DMA is fully contiguous (8KB per partition line).
The natural layout is DMA'd once and stays RESIDENT in SBUF (16MB);
the transposed layout is streamed from HBM in each of the two passes.
HBM traffic per core: 16.8MB (nat, once) + ~21MB of wt tiles; the
other 12 wt tiles are synthesized on-chip by PE transposes of the
resident nat copy, trading idle PE/DVE/Act cycles for DMA bandwidth.
"""

import sys

sys.path.insert(0, "/opt/trn_rl_repo")

import numpy as np

import concourse.bass as bass
import concourse.mybir as mybir
import concourse.tile as tile
from concourse import bacc
from concourse.bass_utils import run_bass_kernel_spmd

BF16 = mybir.dt.bfloat16
F16 = mybir.dt.float16
F32 = mybir.dt.float32
AF = mybir.ActivationFunctionType

B, R, C, OC, IC = 32, 16384, 16, 16, 16
N_CORES = 8
RS = R // N_CORES          # routes per core = 2048
NT = RS // 128             # 128-route tiles per core = 16
OI = OC * IC               # 256
RG = [list(range(N_CORES))]
EXP_SHIFT = 3.0
# tiles per pass whose wt layout is synthesized on-chip (PE transpose of
# resident nat) instead of streamed from HBM -- trades idle PE/Pool/Act
# cycles for DMA bandwidth, the bottleneck.  Pass 3 keeps almost all
# tiles streamed: its transposed tiles hold wt buffers through the
# AllReduce window, blocking stream prefetch.  Values tuned in CoreSim.
N_TP = (1, 1)
TPL = (False, False)   # additionally transpose each pass's final tile,
                     # pre-emitted at t==12 so the PE bubble is absorbed


# ----------------------------------------------------------------- device code

def _squash_stz(nc, pool, v_out, stz_all):
    """v = squash(st / z) computed directly from the packed AllReduce
    result stz_all [64, 136] = [st (c8,o)=128 | z (c8)=8], without forming
    s: squash(st/z) = st * sqrt(n2t) / (z^2 + n2t), n2t = sum_o st^2."""
    st = stz_all[:, 0:128]
    z = stz_all[:, 128:136]
    z2 = pool.tile([64, 8], F32, name="z2", tag="z2")
    nc.vector.tensor_mul(z2[:], z, z)
    sq = pool.tile([64, 128], F32, name="sq", tag="sq")
    nc.vector.tensor_mul(sq[:], st, st)
    n2 = pool.tile([64, 8], F32, name="n2", tag="n2")
    nc.vector.reduce_sum(
        n2[:], sq[:].rearrange("p (c o) -> p c o", o=16), axis=mybir.AxisListType.X
    )
    rt = pool.tile([64, 8], F32, name="rt", tag="rt")
    nc.scalar.activation(rt[:], n2[:], AF.Sqrt)
    d = pool.tile([64, 8], F32, name="d", tag="d")
    nc.vector.tensor_add(d[:], n2[:], z2[:])
    dinv = pool.tile([64, 8], F32, name="dinv", tag="dinv")
    nc.vector.reciprocal(dinv[:], d[:])
    f = pool.tile([64, 8], F32, name="f", tag="f")
    nc.vector.tensor_mul(f[:], rt[:], dinv[:])
    nc.vector.tensor_mul(
        v_out[:].rearrange("p (c o) -> p c o", o=16),
        st.rearrange("p (c o) -> p c o", o=16),
        f[:, :, None].broadcast_to([64, 8, 16]),
    )


def _build_m(nc, small, psum, m_sb, v_sb, s_sel, x2_sb, vt_id_sb):
    """m_sb[128,(c,h,b)=1024] f16 <- M[(o,i),b] = v[b,c,o]*x[b,c,i].

    v_sb [64=(cg,b), 128=(c8,o)] f32.  Transpose v on PE, expand o over i
    via constant selector matmuls, multiply by x replica (x2).
    vexp is built in two 512-col halves to keep the PSUM tag at 2KB.
    """
    vt_ps = psum.tile([128, 64], F32, name="vt_ps", tag="zmb")
    nc.tensor.transpose(vt_ps[:], v_sb[:], vt_id_sb[:])
    vt_sb = small.tile([128, 64], F16, name="vt_sb", tag="vt_sb")
    nc.vector.tensor_copy(vt_sb[:], vt_ps[:])
    for g in range(2):
        vexp_ps = psum.tile([128, 512], F32, name="vexp_ps", tag="zmb")
        for idx in range(16):
            ch = g * 16 + idx
            c = ch // 2
            cg = c // 8
            nc.tensor.matmul(
                vexp_ps[:, idx * 32:idx * 32 + 32],
                s_sel[:, ch * 128:ch * 128 + 128],
                vt_sb[:, cg * 32:cg * 32 + 32],
                start=True, stop=True,
            )
        nc.vector.tensor_mul(
            m_sb[:, g * 512:(g + 1) * 512], vexp_ps[:],
            x2_sb[:, g * 512:(g + 1) * 512])


# Pool/GPSIMD cannot read PSUM on real HW (BIR verification rejects it;
# CoreSim doesn't model the restriction) -- PSUM drains go to DVE/Act only.
_COPY_CYCLE = ("vector", "vector", "vector", "act")


CH_HYB = 2   # chunks of each STREAMED tile transposed instead of DMA'd


def _emit_tp(nc, wtp, tpp, nat_res, id128_sb, t, chunks=None, wt_sb=None):
    """Synthesize wt tile t on-chip: PE transposes of resident nat
    blocks, staged through PSUM, drained to SBUF by DVE/Act."""
    if wt_sb is None:
        wt_sb = wtp.tile([128, 4096], F16, name="wt_sb", tag="wt")
    for chunk in (chunks if chunks is not None else range(16)):
        tp = tpp.tile([128, 256], F16, name="tp", tag="tp")
        for k in range(2):
            ch = chunk * 2 + k
            base = t * 4096 + (ch // 2) * 256 + (ch % 2) * 128
            nc.tensor.transpose(
                tp[:, k * 128:(k + 1) * 128],
                nat_res[:, base:base + 128], id128_sb[:])
        dst = wt_sb[:, chunk * 256:(chunk + 1) * 256]
        eng = _COPY_CYCLE[chunk % len(_COPY_CYCLE)]
        if eng == "pool":
            nc.gpsimd.tensor_copy(dst, tp[:])
        elif eng == "act":
            nc.scalar.activation(dst, tp[:], AF.Copy)
        else:
            nc.vector.tensor_copy(dst, tp[:])
    return wt_sb


def _contract_x(nc, st_out, g_psum, xrep_sb):
    """st_out[64,128] f32 = sum_i g_psum[64,(c8,o,i)=2048] * xrep_sb.

    In-place multiply in PSUM (no SBUF scratch), then free-axis reduce,
    in halves so the final tile's gsum pipeline overlaps this chain.
    """
    nc.vector.tensor_mul(g_psum[:, 0:1024], g_psum[:, 0:1024],
                         xrep_sb[:, 0:1024])
    nc.vector.tensor_mul(g_psum[:, 1024:2048], g_psum[:, 1024:2048],
                         xrep_sb[:, 1024:2048])
    nc.vector.reduce_sum(
        st_out[:, 0:64],
        g_psum[:, 0:1024].rearrange("p (co i) -> p co i", i=16),
        axis=mybir.AxisListType.X,
    )
    nc.vector.reduce_sum(
        st_out[:, 64:128],
        g_psum[:, 1024:2048].rearrange("p (co i) -> p co i", i=16),
        axis=mybir.AxisListType.X,
    )


def build_nc(debug_outputs=False, single_core=False, repeat=1):
    nc = bacc.Bacc("TRN2", target_bir_lowering=False, debug=False,
                   num_devices=1 if single_core else N_CORES)

    wnat = nc.dram_tensor("wnat", [RS, 4096], F16, kind="ExternalInput")
    wt = nc.dram_tensor("wt", [NT, 128, 4096], F16, kind="ExternalInput")
    m0 = nc.dram_tensor("m0", [128, 1024], F16, kind="ExternalInput")
    xrep = nc.dram_tensor("xrep", [64, 2048], F32, kind="ExternalInput")
    x2 = nc.dram_tensor("x2", [128, 1024], F16, kind="ExternalInput")
    smat = nc.dram_tensor("smat", [128, 4096], F16, kind="ExternalInput")
    iden = nc.dram_tensor("iden", [64, 64], F32, kind="ExternalInput")
    id128 = nc.dram_tensor("id128", [128, 128], F16, kind="ExternalInput")
    out = nc.dram_tensor("out", [2, B, 8, OC], F32, kind="ExternalOutput")

    dbg = {}
    if debug_outputs:
        for nm, shp, dt in [("dbg_m0", [128, 1024], F16),
                            ("dbg_p0", [128, 512], F16),
                            ("dbg_st", [64, 128], F32),
                            ("dbg_z", [64, 8], F32)]:
            dbg[nm] = nc.dram_tensor(nm, shp, dt, kind="ExternalOutput")

    with tile.TileContext(nc) as tc:
        _body(nc, tc, wnat, wt, m0, xrep, x2, smat, iden, id128, out, dbg,
              collectives=not single_core, repeat=repeat)
    nc.compile()
    return nc


def _allreduce(nc, ar_out, ar_in, collectives):
    if collectives:
        nc.gpsimd.collective_compute(
            "AllReduce", mybir.AluOpType.add, replica_groups=RG,
            ins=[ar_in.opt()], outs=[ar_out.opt()],
        )
    else:
        nc.sync.dma_start(ar_out[:], ar_in[:])


def _body(nc, tc, wnat, wt, m0, xrep, x2, smat, iden, id128, out, dbg,
          collectives=True, repeat=1):
    with (
        tc.tile_pool(name="pers", bufs=1) as pers,
        tc.tile_pool(name="wtp", bufs=4) as wtp,
        tc.tile_pool(name="small", bufs=1) as small,
        tc.tile_pool(name="psum", bufs=1, space="PSUM") as psum,
        tc.tile_pool(name="pbp", bufs=1, space="PSUM") as pbp,
        tc.tile_pool(name="tpp", bufs=2, space="PSUM") as tpp,
        tc.tile_pool(name="dram", bufs=2, space="DRAM") as dram,
    ):
        # persistent small tensors; the DMAs for tensors not needed until
        # later in the pass are issued inside the rep loop, after the nat
        # prologue, so the first transposes/logits start ASAP.
        xrep_sb = pers.tile([64, 2048], F32)
        x2_sb = pers.tile([128, 1024], F16)
        s_sel = pers.tile([128, 4096], F16)
        id_sb = pers.tile([64, 64], F32)
        id128_sb = pers.tile([128, 128], F16)
        nc.sync.dma_start(id128_sb[:], id128.ap())
        ones_sb = pers.tile([128, 1], F16)
        nc.vector.memset(ones_sb[:], 1.0)
        ones_bb = pers.tile([128, 1], BF16)
        nc.vector.memset(ones_bb[:], 1.0)
        shift_sb = pers.tile([128, 1], F32)
        nc.vector.memset(shift_sb[:], -EXP_SHIFT)
        m0_sb = pers.tile([128, 1024], F16)       # M0 chunks [(c,h) -> 32 cols]
        mc_sb = pers.tile([128, 1024], F16)       # M0+M1 for pass 3
        v_sb = pers.tile([64, 128], F32)          # current v
        nat_res = pers.tile([128, NT * 4096], F16)  # resident natural W

        for _rep in range(repeat):
            late_dmas = [(x2_sb, x2), (s_sel, smat), (id_sb, iden)]

            # ---------------- passes for routing iterations 1 and 2
            for it in range(2):
                ktp = N_TP[it]
                gacc = psum.tile([64, 2048], F32, name="gacc", tag="acc")
                zacc = psum.tile([64, 8], F32, name="zacc", tag="zmb")
                if it == 0:
                    # sources for on-chip-transposed tiles land first; m0
                    # (needed by the first logits, ~6us in) goes after two
                    for t in range(ktp):
                        nc.gpsimd.dma_start(
                            nat_res[:, t * 4096:(t + 1) * 4096],
                            wnat.ap()[t * 128:(t + 1) * 128, :])
                        if t == 1:
                            nc.sync.dma_start(m0_sb[:], m0.ap())
                    if ktp < 2:
                        nc.sync.dma_start(m0_sb[:], m0.ap())
                    if dbg:
                        nc.sync.dma_start(dbg["dbg_m0"].ap(), m0_sb[:])
                    # needed at this pass's contract_x, well after the nats
                    nc.sync.dma_start(xrep_sb[:], xrep.ap())
                tpl = TPL[it]
                if it == 0 and tpl:  # pre-load the final tile's nat source
                    # source for the transposed final tile, right after the
                    # prologue nats so it is resident well before t=12
                    nc.gpsimd.dma_start(
                        nat_res[:, (NT - 1) * 4096:NT * 4096],
                        wnat.ap()[(NT - 1) * 128:NT * 128, :])
                pend = [_emit_tp(nc, wtp, tpp, nat_res, id128_sb, j)
                        for j in range(min(2, ktp))]
                last_tp = None
                for t in range(NT):
                    if t < ktp:
                        wt_sb = pend.pop(0)
                        if t + 2 < ktp:
                            pend.append(
                                _emit_tp(nc, wtp, tpp, nat_res, id128_sb, t + 2))
                    elif tpl and t == NT - 1:
                        wt_sb = last_tp
                    else:
                        wt_sb = wtp.tile([128, 4096], F16, name="wt_sb", tag="wt")
                        nch = 16 - CH_HYB
                        nc.sync.dma_start(wt_sb[:, 0:nch * 256],
                                          wt.ap()[t][:, 0:nch * 256])
                        if it == 0 and not (tpl and t == NT - 1):
                            nc.gpsimd.dma_start(
                                nat_res[:, t * 4096:(t + 1) * 4096],
                                wnat.ap()[t * 128:(t + 1) * 128, :])
                        _emit_tp(nc, wtp, tpp, nat_res, id128_sb, t,
                                 chunks=range(nch, 16), wt_sb=wt_sb)
                    if tpl and t == 12:
                        last_tp = _emit_tp(nc, wtp, tpp, nat_res, id128_sb,
                                           NT - 1)
                    m_cur = m0_sb if it == 0 else mc_sb
                    pb = pbp.tile([128, 512], F32, name="pb", tag="pb")
                    for c in range(16):
                        pcol = ((c % 8) * 2 + c // 8) * 32
                        for h in range(2):
                            off = (c * 2 + h) * 128
                            nc.tensor.matmul(
                                pb[:, pcol:pcol + 32],
                                wt_sb[:, off:off + 128],
                                m_cur[:, (c * 2 + h) * 32:(c * 2 + h) * 32 + 32],
                                start=(h == 0), stop=(h == 1),
                            )
                    p_sb = small.tile([128, 512], F16 if it == 0 else BF16,
                                      name="p_sb", tag="p", bufs=3)
                    nc.scalar.activation(p_sb[:], pb[:], AF.Exp,
                                         bias=shift_sb[:, 0:1])
                    if dbg and it == 0 and t == 0:
                        nc.sync.dma_start(dbg["dbg_p0"].ap(), p_sb[:])
                    # on the final tile, finish gacc's left half (c8 0-3)
                    # first so the contract_x halves pipeline with the tail
                    corder = ([0, 8, 1, 9, 2, 10, 3, 11, 4, 12, 5, 13, 6, 14,
                               7, 15] if t == NT - 1 else range(16))
                    for c in corder:
                        cg, c8 = c // 8, c % 8
                        pcol = (c8 * 2 + cg) * 32
                        nc.tensor.matmul(
                            gacc[cg * 32:(cg + 1) * 32, c8 * 256:(c8 + 1) * 256],
                            p_sb[:, pcol:pcol + 32],
                            nat_res[:, t * 4096 + c * 256:t * 4096 + (c + 1) * 256],
                            start=(t == 0 and c8 % 2 == 0), stop=(t == NT - 1),
                            skip_group_check=True,
                            tile_position=(0, 32 * cg),
                        )
                    for c8 in range(8):
                        nc.tensor.matmul(
                            zacc[:, c8:c8 + 1],
                            p_sb[:, c8 * 64:(c8 + 1) * 64],
                            ones_sb[:, 0:1] if it == 0 else ones_bb[:, 0:1],
                            start=(t == 0 and c8 == 0), stop=(t == NT - 1),
                            skip_group_check=True,
                        )
                # small tensors needed only from the end of pass 2 onward:
                # their DMAs land in the AllReduce dead window
                for sb, dr in late_dmas:
                    nc.sync.dma_start(sb[:], dr.ap())
                late_dmas = []
                # local S~ and Z packed in one [64, 136] stage tile ->
                # one DMA out, one packed AllReduce, one DMA back
                stz = small.tile([64, 136], F32, name="stz", tag="stz")
                _contract_x(nc, stz[:, 0:128], gacc, xrep_sb)
                nc.vector.tensor_copy(stz[:, 128:136], zacc[:])
                if dbg and it == 0:
                    nc.sync.dma_start(dbg["dbg_st"].ap(), stz[:, 0:128])
                    nc.sync.dma_start(dbg["dbg_z"].ap(), stz[:, 128:136])
                arp_in = dram.tile([8704], F32, name="arp_in", tag="arp_in")
                arp_out = dram.tile([8704], F32, name="arp_out", tag="arp_out")
                nc.sync.dma_start(
                    arp_in[:].rearrange("(p f) -> p f", p=64), stz[:]
                )
                _allreduce(nc, arp_out, arp_in, collectives)
                stz_all = small.tile([64, 136], F32, name="stz_all",
                                     tag="stz_all")
                nc.sync.dma_start(
                    stz_all[:], arp_out[:].rearrange("(p f) -> p f", p=64)
                )
                _squash_stz(nc, small, v_sb, stz_all)
                if it == 0:
                    # M1 into mc, then mc += M0: pass-3 logits on M0+M1
                    # give b2 = b1 + u.v1 directly (linearity in M).
                    _build_m(nc, small, psum, mc_sb, v_sb, s_sel, x2_sb, id_sb)
                    nc.vector.tensor_add(mc_sb[:], mc_sb[:], m0_sb[:])
                else:
                    nc.sync.dma_start(
                        out.ap().rearrange("cg b c8 o -> (cg b) (c8 o)"),
                        v_sb[:],
                    )


# ------------------------------------------------------------------ host prep

def _host_inputs(x, W):
    """Per-core input dicts.  x (B,C,IC) f32, W (R,C,OC,IC) f32."""
    x = np.ascontiguousarray(x, dtype=np.float32)
    xb = np.broadcast_to(x[:, :, None, :], (B, C, OC, IC))
    xrep = np.ascontiguousarray(
        xb.reshape(B, 2, 8 * OI).transpose(1, 0, 2).reshape(64, 2048),
        dtype=np.float32)
    xt = x.transpose(2, 1, 0)                      # [i, c, b]
    # x2[p=(po,i), (c,h,b)] = x[b, c, i]  (independent of po and h)
    x2 = np.ascontiguousarray(
        np.broadcast_to(xt[None, :, :, None, :], (8, IC, C, 2, B))
        .reshape(128, 1024).astype(np.float16))
    smat = np.zeros((16, 2, 128, 128), dtype=np.float16)
    pidx = np.arange(128)
    for c in range(16):
        for h in range(2):
            smat[c, h, (c % 8) * 16 + 8 * h + pidx // 16, pidx] = 1.0
    # device wants s_sel[k, (c, h, p)] contiguous
    smat = np.ascontiguousarray(
        smat.transpose(2, 0, 1, 3).reshape(128, 4096))
    iden = np.eye(64, dtype=np.float32)

    # iteration 0 host-side: s0 = x . Wsum / R, v0 = squash(s0), M0 = v0 (x) x
    wsum = W.sum(axis=0, dtype=np.float64)         # (C, OC, IC) exact
    s0 = np.einsum("coi,bci->bco", wsum, x.astype(np.float64)) / R
    n2 = (s0 * s0).sum(-1, keepdims=True)
    v0 = (np.sqrt(n2) / (1.0 + n2)) * s0           # (B, C, OC) f64
    # m0[p=(oh*16+i), (c, h, b)] = v0[b, c, 8h+oh] * x[b, c, i]
    v0r = v0.reshape(B, C, 2, 8)                   # (b, c, h, oh); o = 8h+oh
    m0 = (v0r.transpose(3, 1, 2, 0)[:, None, :, :, :]   # (oh, 1, c, h, b)
          * xt[None, :, :, None, :])               # (1, i, c, 1, b)
    m0 = np.ascontiguousarray(m0.reshape(128, 1024)).astype(np.float16)

    common = dict(m0=m0, xrep=xrep, x2=x2, smat=smat, iden=iden,
                  id128=np.eye(128, dtype=np.float16))
    in_maps = []
    for k in range(N_CORES):
        Ws = np.ascontiguousarray(W[k * RS:(k + 1) * RS], dtype=np.float32)
        wnat = Ws.reshape(RS, 4096).astype(np.float16)
        # wt[t, p, (c, h, q)] = W[r=t*128+q, c, oi=h*128+p]  (contiguous DMA)
        wtk = np.ascontiguousarray(
            Ws.reshape(NT, 128, C, 2, 128).transpose(0, 4, 2, 3, 1)
            .reshape(NT, 128, 4096)).astype(np.float16)
        in_maps.append(dict(wnat=wnat, wt=wtk, **common))
    return in_maps


_NC_CACHE = {}


def _get_nc(debug_outputs=False):
    key = bool(debug_outputs)
    if key not in _NC_CACHE:
        _NC_CACHE[key] = build_nc(debug_outputs)
    return _NC_CACHE[key]


def kernel(x, W):
    nc = _get_nc()
    in_maps = _host_inputs(x, W)
    res = run_bass_kernel_spmd(nc, in_maps, core_ids=list(range(N_CORES)))
    o = res.results[0]["out"]                      # (2, B, 8, OC)
    return np.ascontiguousarray(
        o.transpose(1, 0, 2, 3).reshape(B, C, OC), dtype=np.float32)


if __name__ == "__main__":
    rng = np.random.default_rng(0)
    x = rng.standard_normal((B, C, IC), dtype=np.float32)
    W = rng.standard_normal((R, C, OC, IC), dtype=np.float32)
    out = kernel(x, W)
    print("out", out.shape, out.dtype, np.abs(out).mean())


# revision 95
# speedup vs baseline: 1.1180x; 1.1180x over previous
"""DigitCapsule routing kernel for 8 TRN2 NeuronCores.

Math (reference):
    u_hat[b,r,c,o] = sum_i W[r,c,o,i] x[b,c,i]
    b=0; 3 iterations of: c=softmax_r(b); s=sum_r c*u_hat; v=squash(s);
                          b += sum_o u_hat*v
    returns v (B, C, OC)

Key restructure -- u_hat (536MB) is never materialized.  Each routing
iteration is expressed as matmuls against W directly:
  - logits:  b_inc[b,r,c] = sum_oi W[r,c,oi] * M[b,c,oi],  M = x (x) v
             (contract oi=256; needs W transposed so oi is on partitions)
  - weights: p = exp(b)  (no max subtraction needed, |b| stays small)
  - sums:    G[b,c,oi] = sum_r p[b,r,c] W[r,c,oi]  (contract r; natural W)
             S[b,c,o]  = sum_i G[b,c,o,i] x[b,c,i];  Z[b,c] = sum_r p
  - s = S/Z; v = squash(s)
Iteration 0 has p uniform, so s0/v0/M0 depend only on Wsum = sum_r W --
computed host-side in fp32 (exact) and fed in as the tiny M0 tensor.
The device runs only routing iterations 1 and 2.

Precision: the logits are exponentiated, so every path feeding them must
stay f16-or-better (bf16 W gives |db|~0.03 which compounds to ~30%
output error).

Sharding: R=16384 split over 8 cores (2048 each).  Cross-core traffic is
2 packed AllReduces (34KB each).  W is fed to each core in two layouts
(natural [r, (c,oi)] and transposed [oi, (c,r)]), both prepared
host-side so every DMA is fully contiguous (8KB per partition line).
The natural layout is DMA'd once and stays RESIDENT in SBUF (16MB);
the transposed layout is streamed from HBM in each of the two passes.
HBM traffic per core: 16.8MB (nat, once) + ~21MB of wt tiles; the
other 12 wt tiles are synthesized on-chip by PE transposes of the
resident nat copy, trading idle PE/DVE/Act cycles for DMA bandwidth.
"""

import sys

sys.path.insert(0, "/opt/trn_rl_repo")

import numpy as np

import concourse.bass as bass
import concourse.mybir as mybir
import concourse.tile as tile
from concourse import bacc
from concourse.bass_utils import run_bass_kernel_spmd

BF16 = mybir.dt.bfloat16
F16 = mybir.dt.float16
F32 = mybir.dt.float32
AF = mybir.ActivationFunctionType

B, R, C, OC, IC = 32, 16384, 16, 16, 16
N_CORES = 8
RS = R // N_CORES          # routes per core = 2048
NT = RS // 128             # 128-route tiles per core = 16
OI = OC * IC               # 256
RG = [list(range(N_CORES))]
EXP_SHIFT = 3.0
# tiles per pass whose wt layout is synthesized on-chip (PE transpose of
# resident nat) instead of streamed from HBM -- trades idle PE/Pool/Act
# cycles for DMA bandwidth, the bottleneck.  Pass 3 keeps almost all
# tiles streamed: its transposed tiles hold wt buffers through the
# AllReduce window, blocking stream prefetch.  Values tuned in CoreSim.
N_TP = (1, 1)
TPL = (False, False)   # additionally transpose each pass's final tile,
                     # pre-emitted at t==12 so the PE bubble is absorbed


# ----------------------------------------------------------------- device code

def _squash_stz(nc, pool, v_out, stz_all):
    """v = squash(st / z) computed directly from the packed AllReduce
    result stz_all [64, 136] = [st (c8,o)=128 | z (c8)=8], without forming
    s: squash(st/z) = st * sqrt(n2t) / (z^2 + n2t), n2t = sum_o st^2."""
    st = stz_all[:, 0:128]
    z = stz_all[:, 128:136]
    z2 = pool.tile([64, 8], F32, name="z2", tag="z2")
    nc.vector.tensor_mul(z2[:], z, z)
    sq = pool.tile([64, 128], F32, name="sq", tag="sq")
    nc.vector.tensor_mul(sq[:], st, st)
    n2 = pool.tile([64, 8], F32, name="n2", tag="n2")
    nc.vector.reduce_sum(
        n2[:], sq[:].rearrange("p (c o) -> p c o", o=16), axis=mybir.AxisListType.X
    )
    rt = pool.tile([64, 8], F32, name="rt", tag="rt")
    nc.scalar.activation(rt[:], n2[:], AF.Sqrt)
    d = pool.tile([64, 8], F32, name="d", tag="d")
    nc.vector.tensor_add(d[:], n2[:], z2[:])
    dinv = pool.tile([64, 8], F32, name="dinv", tag="dinv")
    nc.vector.reciprocal(dinv[:], d[:])
    f = pool.tile([64, 8], F32, name="f", tag="f")
    nc.vector.tensor_mul(f[:], rt[:], dinv[:])
    nc.vector.tensor_mul(
        v_out[:].rearrange("p (c o) -> p c o", o=16),
        st.rearrange("p (c o) -> p c o", o=16),
        f[:, :, None].broadcast_to([64, 8, 16]),
    )


def _build_m(nc, small, psum, m_sb, v_sb, s_sel, x2_sb, vt_id_sb):
    """m_sb[128,(c,h,b)=1024] f16 <- M[(o,i),b] = v[b,c,o]*x[b,c,i].

    v_sb [64=(cg,b), 128=(c8,o)] f32.  Transpose v on PE, expand o over i
    via constant selector matmuls, multiply by x replica (x2).
    vexp is built in two 512-col halves to keep the PSUM tag at 2KB.
    """
    vt_ps = psum.tile([128, 64], F32, name="vt_ps", tag="zmb")
    nc.tensor.transpose(vt_ps[:], v_sb[:], vt_id_sb[:])
    vt_sb = small.tile([128, 64], F16, name="vt_sb", tag="vt_sb")
    nc.vector.tensor_copy(vt_sb[:], vt_ps[:])
    for g in range(2):
        vexp_ps = psum.tile([128, 512], F32, name="vexp_ps", tag="zmb")
        for idx in range(16):
            ch = g * 16 + idx
            c = ch // 2
            cg = c // 8
            nc.tensor.matmul(
                vexp_ps[:, idx * 32:idx * 32 + 32],
                s_sel[:, ch * 128:ch * 128 + 128],
                vt_sb[:, cg * 32:cg * 32 + 32],
                start=True, stop=True,
            )
        nc.vector.tensor_mul(
            m_sb[:, g * 512:(g + 1) * 512], vexp_ps[:],
            x2_sb[:, g * 512:(g + 1) * 512])


# Pool/GPSIMD cannot read PSUM on real HW (BIR verification rejects it;
# CoreSim doesn't model the restriction) -- PSUM drains go to DVE/Act only.
_COPY_CYCLE = ("vector", "vector", "vector", "act")


CH_HYB = (2, 1)  # per-pass: chunks of each streamed tile transposed


def _emit_tp(nc, wtp, tpp, nat_res, id128_sb, t, chunks=None, wt_sb=None):
    """Synthesize wt tile t on-chip: PE transposes of resident nat
    blocks, staged through PSUM, drained to SBUF by DVE/Act."""
    if wt_sb is None:
        wt_sb = wtp.tile([128, 4096], F16, name="wt_sb", tag="wt")
    for chunk in (chunks if chunks is not None else range(16)):
        tp = tpp.tile([128, 256], F16, name="tp", tag="tp")
        for k in range(2):
            ch = chunk * 2 + k
            base = t * 4096 + (ch // 2) * 256 + (ch % 2) * 128
            nc.tensor.transpose(
                tp[:, k * 128:(k + 1) * 128],
                nat_res[:, base:base + 128], id128_sb[:])
        dst = wt_sb[:, chunk * 256:(chunk + 1) * 256]
        eng = _COPY_CYCLE[chunk % len(_COPY_CYCLE)]
        if eng == "pool":
            nc.gpsimd.tensor_copy(dst, tp[:])
        elif eng == "act":
            nc.scalar.activation(dst, tp[:], AF.Copy)
        else:
            nc.vector.tensor_copy(dst, tp[:])
    return wt_sb


def _contract_x(nc, st_out, g_psum, xrep_sb):
    """st_out[64,128] f32 = sum_i g_psum[64,(c8,o,i)=2048] * xrep_sb.

    In-place multiply in PSUM (no SBUF scratch), then free-axis reduce,
    in halves so the final tile's gsum pipeline overlaps this chain.
    """
    nc.vector.tensor_mul(g_psum[:, 0:1024], g_psum[:, 0:1024],
                         xrep_sb[:, 0:1024])
    nc.vector.tensor_mul(g_psum[:, 1024:2048], g_psum[:, 1024:2048],
                         xrep_sb[:, 1024:2048])
    nc.vector.reduce_sum(
        st_out[:, 0:64],
        g_psum[:, 0:1024].rearrange("p (co i) -> p co i", i=16),
        axis=mybir.AxisListType.X,
    )
    nc.vector.reduce_sum(
        st_out[:, 64:128],
        g_psum[:, 1024:2048].rearrange("p (co i) -> p co i", i=16),
        axis=mybir.AxisListType.X,
    )


def build_nc(debug_outputs=False, single_core=False, repeat=1):
    nc = bacc.Bacc("TRN2", target_bir_lowering=False, debug=False,
                   num_devices=1 if single_core else N_CORES)

    wnat = nc.dram_tensor("wnat", [RS, 4096], F16, kind="ExternalInput")
    wt = nc.dram_tensor("wt", [NT, 128, 4096], F16, kind="ExternalInput")
    m0 = nc.dram_tensor("m0", [128, 1024], F16, kind="ExternalInput")
    xrep = nc.dram_tensor("xrep", [64, 2048], F32, kind="ExternalInput")
    x2 = nc.dram_tensor("x2", [128, 1024], F16, kind="ExternalInput")
    smat = nc.dram_tensor("smat", [128, 4096], F16, kind="ExternalInput")
    iden = nc.dram_tensor("iden", [64, 64], F32, kind="ExternalInput")
    id128 = nc.dram_tensor("id128", [128, 128], F16, kind="ExternalInput")
    out = nc.dram_tensor("out", [2, B, 8, OC], F32, kind="ExternalOutput")

    dbg = {}
    if debug_outputs:
        for nm, shp, dt in [("dbg_m0", [128, 1024], F16),
                            ("dbg_p0", [128, 512], F16),
                            ("dbg_st", [64, 128], F32),
                            ("dbg_z", [64, 8], F32)]:
            dbg[nm] = nc.dram_tensor(nm, shp, dt, kind="ExternalOutput")

    with tile.TileContext(nc) as tc:
        _body(nc, tc, wnat, wt, m0, xrep, x2, smat, iden, id128, out, dbg,
              collectives=not single_core, repeat=repeat)
    nc.compile()
    return nc


def _allreduce(nc, ar_out, ar_in, collectives):
    if collectives:
        nc.gpsimd.collective_compute(
            "AllReduce", mybir.AluOpType.add, replica_groups=RG,
            ins=[ar_in.opt()], outs=[ar_out.opt()],
        )
    else:
        nc.sync.dma_start(ar_out[:], ar_in[:])


def _body(nc, tc, wnat, wt, m0, xrep, x2, smat, iden, id128, out, dbg,
          collectives=True, repeat=1):
    with (
        tc.tile_pool(name="pers", bufs=1) as pers,
        tc.tile_pool(name="wtp", bufs=4) as wtp,
        tc.tile_pool(name="small", bufs=1) as small,
        tc.tile_pool(name="psum", bufs=1, space="PSUM") as psum,
        tc.tile_pool(name="pbp", bufs=1, space="PSUM") as pbp,
        tc.tile_pool(name="tpp", bufs=2, space="PSUM") as tpp,
        tc.tile_pool(name="dram", bufs=2, space="DRAM") as dram,
    ):
        # persistent small tensors; the DMAs for tensors not needed until
        # later in the pass are issued inside the rep loop, after the nat
        # prologue, so the first transposes/logits start ASAP.
        xrep_sb = pers.tile([64, 2048], F32)
        x2_sb = pers.tile([128, 1024], F16)
        s_sel = pers.tile([128, 4096], F16)
        id_sb = pers.tile([64, 64], F32)
        id128_sb = pers.tile([128, 128], F16)
        nc.sync.dma_start(id128_sb[:], id128.ap())
        ones_sb = pers.tile([128, 1], F16)
        nc.vector.memset(ones_sb[:], 1.0)
        ones_bb = pers.tile([128, 1], BF16)
        nc.vector.memset(ones_bb[:], 1.0)
        shift_sb = pers.tile([128, 1], F32)
        nc.vector.memset(shift_sb[:], -EXP_SHIFT)
        m0_sb = pers.tile([128, 1024], F16)       # M0 chunks [(c,h) -> 32 cols]
        mc_sb = pers.tile([128, 1024], F16)       # M0+M1 for pass 3
        v_sb = pers.tile([64, 128], F32)          # current v
        nat_res = pers.tile([128, NT * 4096], F16)  # resident natural W

        for _rep in range(repeat):
            late_dmas = [(x2_sb, x2), (s_sel, smat), (id_sb, iden)]

            # ---------------- passes for routing iterations 1 and 2
            for it in range(2):
                ktp = N_TP[it]
                gacc = psum.tile([64, 2048], F32, name="gacc", tag="acc")
                zacc = psum.tile([64, 8], F32, name="zacc", tag="zmb")
                if it == 0:
                    # sources for on-chip-transposed tiles land first; m0
                    # (needed by the first logits, ~6us in) goes after two
                    for t in range(ktp):
                        nc.gpsimd.dma_start(
                            nat_res[:, t * 4096:(t + 1) * 4096],
                            wnat.ap()[t * 128:(t + 1) * 128, :])
                        if t == 1:
                            nc.sync.dma_start(m0_sb[:], m0.ap())
                    if ktp < 2:
                        nc.sync.dma_start(m0_sb[:], m0.ap())
                    if dbg:
                        nc.sync.dma_start(dbg["dbg_m0"].ap(), m0_sb[:])
                    # needed at this pass's contract_x, well after the nats
                    nc.sync.dma_start(xrep_sb[:], xrep.ap())
                tpl = TPL[it]
                if it == 0 and tpl:  # pre-load the final tile's nat source
                    # source for the transposed final tile, right after the
                    # prologue nats so it is resident well before t=12
                    nc.gpsimd.dma_start(
                        nat_res[:, (NT - 1) * 4096:NT * 4096],
                        wnat.ap()[(NT - 1) * 128:NT * 128, :])
                pend = [_emit_tp(nc, wtp, tpp, nat_res, id128_sb, j)
                        for j in range(min(2, ktp))]
                last_tp = None
                for t in range(NT):
                    if t < ktp:
                        wt_sb = pend.pop(0)
                        if t + 2 < ktp:
                            pend.append(
                                _emit_tp(nc, wtp, tpp, nat_res, id128_sb, t + 2))
                    elif tpl and t == NT - 1:
                        wt_sb = last_tp
                    else:
                        wt_sb = wtp.tile([128, 4096], F16, name="wt_sb", tag="wt")
                        nch = 16 - CH_HYB[it]
                        nc.sync.dma_start(wt_sb[:, 0:nch * 256],
                                          wt.ap()[t][:, 0:nch * 256])
                        if it == 0 and not (tpl and t == NT - 1):
                            nc.gpsimd.dma_start(
                                nat_res[:, t * 4096:(t + 1) * 4096],
                                wnat.ap()[t * 128:(t + 1) * 128, :])
                        _emit_tp(nc, wtp, tpp, nat_res, id128_sb, t,
                                 chunks=range(nch, 16), wt_sb=wt_sb)
                    if tpl and t == 12:
                        last_tp = _emit_tp(nc, wtp, tpp, nat_res, id128_sb,
                                           NT - 1)
                    m_cur = m0_sb if it == 0 else mc_sb
                    pb = pbp.tile([128, 512], F32, name="pb", tag="pb")
                    for c in range(16):
                        pcol = ((c % 8) * 2 + c // 8) * 32
                        for h in range(2):
                            off = (c * 2 + h) * 128
                            nc.tensor.matmul(
                                pb[:, pcol:pcol + 32],
                                wt_sb[:, off:off + 128],
                                m_cur[:, (c * 2 + h) * 32:(c * 2 + h) * 32 + 32],
                                start=(h == 0), stop=(h == 1),
                            )
                    p_sb = small.tile([128, 512], F16 if it == 0 else BF16,
                                      name="p_sb", tag="p", bufs=3)
                    nc.scalar.activation(p_sb[:], pb[:], AF.Exp,
                                         bias=shift_sb[:, 0:1])
                    if dbg and it == 0 and t == 0:
                        nc.sync.dma_start(dbg["dbg_p0"].ap(), p_sb[:])
                    # on the final tile, finish gacc's left half (c8 0-3)
                    # first so the contract_x halves pipeline with the tail
                    corder = ([0, 8, 1, 9, 2, 10, 3, 11, 4, 12, 5, 13, 6, 14,
                               7, 15] if t == NT - 1 else range(16))
                    for c in corder:
                        cg, c8 = c // 8, c % 8
                        pcol = (c8 * 2 + cg) * 32
                        nc.tensor.matmul(
                            gacc[cg * 32:(cg + 1) * 32, c8 * 256:(c8 + 1) * 256],
                            p_sb[:, pcol:pcol + 32],
                            nat_res[:, t * 4096 + c * 256:t * 4096 + (c + 1) * 256],
                            start=(t == 0 and c8 % 2 == 0), stop=(t == NT - 1),
                            skip_group_check=True,
                            tile_position=(0, 32 * cg),
                        )
                    for c8 in range(8):
                        nc.tensor.matmul(
                            zacc[:, c8:c8 + 1],
                            p_sb[:, c8 * 64:(c8 + 1) * 64],
                            ones_sb[:, 0:1] if it == 0 else ones_bb[:, 0:1],
                            start=(t == 0 and c8 == 0), stop=(t == NT - 1),
                            skip_group_check=True,
                        )
                # small tensors needed only from the end of pass 2 onward:
                # their DMAs land in the AllReduce dead window
                for sb, dr in late_dmas:
                    nc.sync.dma_start(sb[:], dr.ap())
                late_dmas = []
                # local S~ and Z packed in one [64, 136] stage tile ->
                # one DMA out, one packed AllReduce, one DMA back
                stz = small.tile([64, 136], F32, name="stz", tag="stz")
                _contract_x(nc, stz[:, 0:128], gacc, xrep_sb)
                nc.vector.tensor_copy(stz[:, 128:136], zacc[:])
                if dbg and it == 0:
                    nc.sync.dma_start(dbg["dbg_st"].ap(), stz[:, 0:128])
                    nc.sync.dma_start(dbg["dbg_z"].ap(), stz[:, 128:136])
                arp_in = dram.tile([8704], F32, name="arp_in", tag="arp_in")
                arp_out = dram.tile([8704], F32, name="arp_out", tag="arp_out")
                nc.sync.dma_start(
                    arp_in[:].rearrange("(p f) -> p f", p=64), stz[:]
                )
                _allreduce(nc, arp_out, arp_in, collectives)
                stz_all = small.tile([64, 136], F32, name="stz_all",
                                     tag="stz_all")
                nc.sync.dma_start(
                    stz_all[:], arp_out[:].rearrange("(p f) -> p f", p=64)
                )
                _squash_stz(nc, small, v_sb, stz_all)
                if it == 0:
                    # M1 into mc, then mc += M0: pass-3 logits on M0+M1
                    # give b2 = b1 + u.v1 directly (linearity in M).
                    _build_m(nc, small, psum, mc_sb, v_sb, s_sel, x2_sb, id_sb)
                    nc.vector.tensor_add(mc_sb[:], mc_sb[:], m0_sb[:])
                else:
                    nc.sync.dma_start(
                        out.ap().rearrange("cg b c8 o -> (cg b) (c8 o)"),
                        v_sb[:],
                    )


# ------------------------------------------------------------------ host prep

def _host_inputs(x, W):
    """Per-core input dicts.  x (B,C,IC) f32, W (R,C,OC,IC) f32."""
    x = np.ascontiguousarray(x, dtype=np.float32)
    xb = np.broadcast_to(x[:, :, None, :], (B, C, OC, IC))
    xrep = np.ascontiguousarray(
        xb.reshape(B, 2, 8 * OI).transpose(1, 0, 2).reshape(64, 2048),
        dtype=np.float32)
    xt = x.transpose(2, 1, 0)                      # [i, c, b]
    # x2[p=(po,i), (c,h,b)] = x[b, c, i]  (independent of po and h)
    x2 = np.ascontiguousarray(
        np.broadcast_to(xt[None, :, :, None, :], (8, IC, C, 2, B))
        .reshape(128, 1024).astype(np.float16))
    smat = np.zeros((16, 2, 128, 128), dtype=np.float16)
    pidx = np.arange(128)
    for c in range(16):
        for h in range(2):
            smat[c, h, (c % 8) * 16 + 8 * h + pidx // 16, pidx] = 1.0
    # device wants s_sel[k, (c, h, p)] contiguous
    smat = np.ascontiguousarray(
        smat.transpose(2, 0, 1, 3).reshape(128, 4096))
    iden = np.eye(64, dtype=np.float32)

    # iteration 0 host-side: s0 = x . Wsum / R, v0 = squash(s0), M0 = v0 (x) x
    wsum = W.sum(axis=0, dtype=np.float64)         # (C, OC, IC) exact
    s0 = np.einsum("coi,bci->bco", wsum, x.astype(np.float64)) / R
    n2 = (s0 * s0).sum(-1, keepdims=True)
    v0 = (np.sqrt(n2) / (1.0 + n2)) * s0           # (B, C, OC) f64
    # m0[p=(oh*16+i), (c, h, b)] = v0[b, c, 8h+oh] * x[b, c, i]
    v0r = v0.reshape(B, C, 2, 8)                   # (b, c, h, oh); o = 8h+oh
    m0 = (v0r.transpose(3, 1, 2, 0)[:, None, :, :, :]   # (oh, 1, c, h, b)
          * xt[None, :, :, None, :])               # (1, i, c, 1, b)
    m0 = np.ascontiguousarray(m0.reshape(128, 1024)).astype(np.float16)

    common = dict(m0=m0, xrep=xrep, x2=x2, smat=smat, iden=iden,
                  id128=np.eye(128, dtype=np.float16))
    in_maps = []
    for k in range(N_CORES):
        Ws = np.ascontiguousarray(W[k * RS:(k + 1) * RS], dtype=np.float32)
        wnat = Ws.reshape(RS, 4096).astype(np.float16)
        # wt[t, p, (c, h, q)] = W[r=t*128+q, c, oi=h*128+p]  (contiguous DMA)
        wtk = np.ascontiguousarray(
            Ws.reshape(NT, 128, C, 2, 128).transpose(0, 4, 2, 3, 1)
            .reshape(NT, 128, 4096)).astype(np.float16)
        in_maps.append(dict(wnat=wnat, wt=wtk, **common))
    return in_maps


_NC_CACHE = {}


def _get_nc(debug_outputs=False):
    key = bool(debug_outputs)
    if key not in _NC_CACHE:
        _NC_CACHE[key] = build_nc(debug_outputs)
    return _NC_CACHE[key]


def kernel(x, W):
    nc = _get_nc()
    in_maps = _host_inputs(x, W)
    res = run_bass_kernel_spmd(nc, in_maps, core_ids=list(range(N_CORES)))
    o = res.results[0]["out"]                      # (2, B, 8, OC)
    return np.ascontiguousarray(
        o.transpose(1, 0, 2, 3).reshape(B, C, OC), dtype=np.float32)


if __name__ == "__main__":
    rng = np.random.default_rng(0)
    x = rng.standard_normal((B, C, IC), dtype=np.float32)
    W = rng.standard_normal((R, C, OC, IC), dtype=np.float32)
    out = kernel(x, W)
    print("out", out.shape, out.dtype, np.abs(out).mean())
